# revision 58
# baseline (speedup 1.0000x reference)
"""CNF vector field + exact divergence kernel for Trainium2 (8 NeuronCores).

Math (per sample x of dim D=64, t scalar, 3-layer MLP 65->512->512->64):
    h1 = tanh(W1^T [t;x] + b1)
    h2 = tanh(W2^T h1 + b2)
    dx = W3^T h2 + b3
    div = trace(d dx / d x) collapses to the bilinear form
        div = (1-h1^2)^T G (1-h2^2)   with  G = W2 * (W1[1:].T @ W3.T)

Sharding: pure data parallel over batch; 8192/8 = 1024 samples per core;
weights replicated.  Everything on device is feature-major ([feature
partitions, batch free]): the host pre-transposes x (and appends t/ones
rows), precomputes G, and re-transposes the feature-major output, so the
device program is matmul/activation/elementwise only — no PE transposes,
no identity, no PSUM round-trips beyond the matmul outputs themselves.

Per-core device work (CH=batch chunk):
    L1  : h1 = tanh(W1hat^T xhat)            (b1/t folded into xhat rows)
    L2  : h2 = tanh(W2^T h1 + b2)
    c   : c  = G^T s1m,  s1m = (h1-1)(h1+1) = -(1-h1^2)
    e   : e  = (s2q-1) * c = (1-h2^2) * (G^T (1-h1^2)),  s2q = h2^2
    L3  : out[0:64] = W3^T h2 (+b3);  out[64] = ones^T e = div
    out : one [65, CH] DMA per chunk, host transposes back
"""

import sys

if "/opt/trn_rl_repo" not in sys.path:
    sys.path.insert(0, "/opt/trn_rl_repo")

import numpy as np

D = 64
H = 512
B = 8192
N_CORES = 8
BC = B // N_CORES          # 1024 samples per core
NCH = 4                    # batch chunks per core (pipeline granularity)
CH = BC // NCH             # 256 moving columns per matmul
KT = H // 128              # 4 k-tiles of the hidden dim
N_WARM = 18                # PE warm-up matmuls during the DMA head (~3.8us
                           # sustained at the cold clock, to flip HAM to 8/8)
# A=L1, B=L2, C=c/ebuf, L=L3, V=div+act+dma  (per chunk 0-3)
SCHEDULE = [
    "A0", "A1", "B0", "A2", "B1", "A3", "L0", "C0", "B2", "L1",
    "C1", "V0", "B3", "L2", "C2", "V1", "C3", "L3", "V2", "V3",
]

_CACHE = {}


def _patch_tile_drain():
    """walrus in this toolchain accepts only one sync wait per CTRL
    instruction; split the TileContext tail-drain waits across nops."""
    import concourse.mybir as mybir
    from concourse.tile import TileContext
    from concourse.vector_clock import ScopedClock

    if getattr(TileContext, "_drain_patched", False):
        return

    def _drain_and_barrier(self, tick_clock, wait_clock):
        nc = self.nc
        probe = nc.sync.nop(nofuse=True, hint="drain_wait_probe")
        wait_clock.add_sem_waits(
            probe.ins, ScopedClock({None: tick_clock.global_clock})
        )
        waits = list(probe.ins.sync_info.on_wait) if probe.ins.sync_info else []
        if len(waits) > 1:
            probe.ins.sync_info.on_wait.clear()
            probe.ins.sync_info.on_wait.append(waits[0])
            for w in waits[1:]:
                nop_inst = nc.sync.nop(nofuse=True, hint="drain_wait_split")
                if nop_inst.ins.sync_info is None:
                    nop_inst.ins.sync_info = mybir.SyncInfo(on_wait=[], on_update=[])
                nop_inst.ins.sync_info.on_wait.append(w)
        nc.sync.drain()  # SP already observed every sem above
        popped = nc._tile_sem_poison_stack.pop()
        assert popped is self._sem_poison
        # sem clears skipped: NRT reloads sem state per execution; verified
        # by repeated-call correctness checks in test.py

    TileContext._orig_drain_and_barrier = TileContext._drain_and_barrier
    TileContext._drain_and_barrier = _drain_and_barrier
    TileContext._drain_patched = True


# this walrus build has small per-instruction sync-wait budgets; split any
# excess waits onto same-engine nops placed just before the instruction
# (waiting earlier on the same engine stream is always safe).
_WAIT_LIMITS = {"DMACOPY": 1, "NOOP": 1, "DRAIN": 1, "TRIGGEREDCOPY": 1}
_DEFAULT_WAIT_LIMIT = 1


def _split_excess_waits(nc):
    import concourse.mybir as mybir

    ctr = 0
    for fn in nc.m.functions:
        for blk in fn.blocks:
            lst = blk.instructions
            out = []
            changed = False
            for inst in lst:
                si = inst.sync_info
                waits = list(si.on_wait) if si else []
                opname = type(inst).__name__.replace("Inst", "").upper()
                limit = _WAIT_LIMITS.get(opname, _DEFAULT_WAIT_LIMIT)
                if len(waits) > limit:
                    keep = waits[-limit:]
                    excess = waits[:-limit]
                    si.on_wait.clear()
                    for w in keep:
                        si.on_wait.append(w)
                    for w in excess:
                        nop = mybir.InstNoOp(name=f"WSPLIT-{ctr}", ins=[], outs=[])
                        ctr += 1
                        nop.engine = inst.engine
                        nop.sync_info = mybir.SyncInfo(on_wait=[w], on_update=[])
                        out.append(nop)
                    changed = True
                out.append(inst)
            if changed:
                lst[:] = out


def _build(for_sim=False):
    import concourse.bass as bass
    import concourse.mybir as mybir
    from concourse.tile import TileContext

    _patch_tile_drain()

    f32 = mybir.dt.float32
    bf16 = mybir.dt.bfloat16
    f8 = mybir.dt.float8e4
    AF = mybir.ActivationFunctionType
    OP = mybir.AluOpType

    nc = bass.Bass(trn_type="TRN2")
    nc._bass_sim_build = for_sim

    # host-prepped inputs (see kernel()); all matmul operands in bf16
    # (fp32 PSUM accumulation keeps the error ~0.4%, well under the 2e-2
    # budget), fp32 only for biases and the final output:
    #   xw_d  : [66, 512+BC]  cols 0:512 = W1hat (rows: W1[1:], W1[0], b1),
    #                         cols 512:  = x^T with t/ones rows appended
    #   wgw_d : [128, 4*512 + 4*576]  = W2 tiles, then [G | W3] tiles,
    #                         each [p, t, m] flattened (p=128, t=4)
    #   b3a   : [65, 1]       rows 0:64 = b3, row 64 = 0
    xw_d = nc.dram_tensor("xw_d", [D + 2, H + BC], bf16, kind="ExternalInput")
    wgw_d = nc.dram_tensor(
        "wgw_d", [128, KT * H + KT * (H + D)], bf16, kind="ExternalInput"
    )
    b2v = nc.dram_tensor("b2v", [H], f32, kind="ExternalInput")
    b3a = nc.dram_tensor("b3a", [D + 1, 1], f32, kind="ExternalInput")
    out_t = nc.dram_tensor("out_t", [D + 1, BC], f32, kind="ExternalOutput")

    with TileContext(nc) as tc:
        with (
            tc.tile_pool(name="weights", bufs=1) as wpool,
            tc.tile_pool(name="acts", bufs=1) as apool,
            tc.tile_pool(name="psmm", bufs=6, space="PSUM") as psmm,
            tc.tile_pool(name="pso", bufs=2, space="PSUM") as pso,
        ):
            ones_col = wpool.tile([128, 1], bf16)
            nc.vector.memset(ones_col, 1.0)
            warm_mv = wpool.tile([128, 256], bf16)
            nc.vector.memset(warm_mv, 0.0)

            # ---------------- input / weight DMAs --------------------------
            # strict need-order on one queue so each transfer gets the full
            # parallel DMA-engine bandwidth: xw (L1) -> w24 (L2) -> g4/w34 (c).
            # A concurrent w24 on the other queue delays xw past the warm-up
            # dummies and the resulting PE gap re-throttles HAM.
            xw = apool.tile([D + 2, H + BC], bf16)
            nc.sync.dma_start(out=xw, in_=xw_d[:])
            w1hat = xw[:, 0:H]
            xhat = xw[:, H : H + BC]
            wgw = wpool.tile([128, KT, 2 * H + D], bf16)
            nc.sync.dma_start(
                out=wgw[:, :, 0:H],
                in_=wgw_d[:, 0 : KT * H].rearrange("p (t m) -> p t m", t=KT),
            )
            nc.sync.dma_start(
                out=wgw[:, :, H:],
                in_=wgw_d[:, KT * H :].rearrange("p (t m) -> p t m", t=KT),
            )
            w24 = wgw[:, :, 0:H]
            g4 = wgw[:, :, H : 2 * H]
            w34 = wgw[:, :, 2 * H : 2 * H + D]
            b2t = wpool.tile([128, KT], f32)
            nc.gpsimd.dma_start(out=b2t, in_=b2v[:].rearrange("(t p) -> p t", p=128))
            b3t = wpool.tile([D + 1, 1], f32)
            nc.gpsimd.dma_start(out=b3t, in_=b3a[:])

            # ---------------- PE warm-up during the DMA head ---------------
            # HAM un-throttles the PE clock only after ~3.4us of sustained
            # activity; burn the DMA wait on dummy matmuls so the real ones
            # start at full rate.
            pwarm = psmm.tile([1, 256], f32, tag="mmtile", name="pwarm")
            for _ in range(N_WARM):
                nc.tensor.matmul(pwarm, ones_col, warm_mv, start=True, stop=True)

            # per-chunk activation tiles (bf16 matmul operands)
            h1 = [apool.tile([128, KT, CH], bf16, tag=f"h1_{n}", name=f"h1_{n}") for n in range(NCH)]
            u1 = [apool.tile([128, KT, CH], bf16, tag=f"u1_{n}", name=f"u1_{n}") for n in range(NCH)]
            s1m = [apool.tile([128, KT, CH], bf16, tag=f"s1m{n}", name=f"s1m{n}") for n in range(NCH)]
            h2 = [apool.tile([128, KT, CH], bf16, tag=f"h2_{n}", name=f"h2_{n}") for n in range(NCH)]
            s2q = [apool.tile([128, KT, CH], bf16, tag=f"s2q{n}", name=f"s2q{n}") for n in range(NCH)]
            ebuf = [apool.tile([128, KT, CH], bf16, tag=f"eb{n}", name=f"eb{n}") for n in range(NCH)]

            # ---- stage emitters (per chunk) -------------------------------
            def stage_l1(n):
                # h1 = tanh(W1hat^T xhat); u1 = h1+1; s1m = (h1-1)(h1+1)
                xh_n = xhat[:, n * CH : (n + 1) * CH]
                for i in range(KT):
                    pz = psmm.tile([128, CH], f32, tag="mmtile")
                    nc.tensor.matmul(
                        pz,
                        w1hat[:, i * 128 : (i + 1) * 128],
                        xh_n,
                        start=True,
                        stop=True,
                    )
                    nc.scalar.activation(h1[n][:, i, :], pz, AF.Tanh)
                    nc.vector.tensor_scalar_add(u1[n][:, i, :], h1[n][:, i, :], 1.0)
                    nc.vector.scalar_tensor_tensor(
                        out=s1m[n][:, i, :],
                        in0=h1[n][:, i, :],
                        scalar=1.0,
                        in1=u1[n][:, i, :],
                        op0=OP.subtract,
                        op1=OP.mult,
                    )

            def stage_l2(n, k_outer=False):
                # h2 = tanh(W2^T h1 + b2); s2q = h2^2
                # k_outer: first 4 matmuls need only h1[k=0] — pipelines with
                # the serialized tanh chain during the L1(0) fill phase.
                pzs = [
                    psmm.tile([128, CH], f32, tag="mmtile", name=f"pz{n}_{i}")
                    for i in range(KT)
                ]
                loop = (
                    [(i, k) for k in range(KT) for i in range(KT)]
                    if k_outer
                    else [(i, k) for i in range(KT) for k in range(KT)]
                )
                for i, k in loop:
                    nc.tensor.matmul(
                        pzs[i],
                        w24[:, k, i * 128 : (i + 1) * 128],
                        h1[n][:, k, :],
                        start=(k == 0),
                        stop=(k == KT - 1),
                    )
                    if k == KT - 1:
                        nc.scalar.activation(
                            h2[n][:, i, :], pzs[i], AF.Tanh, bias=b2t[:, i : i + 1]
                        )
                        nc.gpsimd.tensor_mul(
                            s2q[n][:, i, :], h2[n][:, i, :], h2[n][:, i, :]
                        )

            def stage_c(n):
                # c = G^T s1m; e = (s2q - 1) * c
                for i in range(KT):
                    pc = psmm.tile([128, CH], f32, tag="mmtile")
                    for k in range(KT):
                        nc.tensor.matmul(
                            pc,
                            g4[:, k, i * 128 : (i + 1) * 128],
                            s1m[n][:, k, :],
                            start=(k == 0),
                            stop=(k == KT - 1),
                        )
                    # gpsimd cannot read PSUM; ebuf stays on DVE
                    nc.vector.scalar_tensor_tensor(
                        out=ebuf[n][:, i, :],
                        in0=s2q[n][:, i, :],
                        scalar=1.0,
                        in1=pc,
                        op0=OP.subtract,
                        op1=OP.mult,
                    )

            # po tiles live from stage_l3 (L3 matmuls, needs only h2) until
            # stage_div (div matmuls + act + DMA, needs ebuf)
            po_t = [None] * NCH
            ott = apool.tile([D + 1, BC], f32, name="ott")

            def stage_l3(n):
                # out[0:64] = W3^T h2 (b3 added by the Identity act later)
                po = pso.tile([D + 1, CH], f32, tag="po", name=f"po{n}")
                po_t[n] = po
                for k in range(KT):
                    nc.tensor.matmul(
                        po[0:D, :],
                        w34[:, k, :],
                        h2[n][:, k, :],
                        start=(k == 0),
                        stop=(k == KT - 1),
                    )

            def stage_div(n):
                # out[64] = ones^T e, then one act + one DMA out
                po = po_t[n]
                for k in range(KT):
                    nc.tensor.matmul(
                        po[D : D + 1, :],
                        ones_col,
                        ebuf[n][:, k, :],
                        start=(k == 0),
                        stop=(k == KT - 1),
                    )
                nc.scalar.activation(ott[:, n * CH : (n + 1) * CH], po, AF.Identity, bias=b3t)
                if n % 2 == 1:
                    # one DMA per chunk pair, alternating HWDGE queues so the
                    # last two dispatches overlap instead of serializing
                    dq = nc.scalar if n == 1 else nc.sync
                    lo, hi = (n - 1) * CH, (n + 1) * CH
                    dq.dma_start(out=out_t[:, lo:hi], in_=ott[:, lo:hi])

            # ---- software-pipeline schedule over chunks -------------------
            # Explicit order; each engine's stream interleaves across chunks
            # so no serialized elementwise stream gates a later consumer.
            # A=L1, B=L2, C=c/ebuf, L=L3, V=div+act+dma.
            emit = {
                "A": stage_l1,
                "B": stage_l2,
                "C": stage_c,
                "L": stage_l3,
                "V": stage_div,
            }
            for step in SCHEDULE:
                if step[0] == "W":
                    # filler dummies: keep PE busy/warm while a DMA lands
                    for _ in range(int(step[1:])):
                        nc.tensor.matmul(
                            pwarm, ones_col, warm_mv, start=True, stop=True
                        )
                elif step == "B0":
                    stage_l2(0, k_outer=True)
                else:
                    emit[step[0]](int(step[1]))

    if not for_sim:
        _split_excess_waits(nc)
    return nc


def _get_nc():
    if "nc" not in _CACHE:
        _CACHE["nc"] = _build()
    return _CACHE["nc"]


def _make_in_maps(t, x, W1, b1, W2, b2, W3, b3):
    t = np.asarray(t, np.float32)
    x = np.asarray(x, np.float32)
    W1 = np.asarray(W1, np.float32)
    b1 = np.asarray(b1, np.float32)
    W2 = np.asarray(W2, np.float32)
    b2 = np.asarray(b2, np.float32)
    W3 = np.asarray(W3, np.float32)
    b3 = np.asarray(b3, np.float32)

    import ml_dtypes

    bf16 = ml_dtypes.bfloat16
    fp8 = ml_dtypes.float8_e4m3

    # feature-major input with t/ones rows appended: [66, B]
    x_t_full = np.empty((D + 2, B), np.float32)
    x_t_full[0:D] = x[:, 0:D].T
    x_t_full[D] = t[0]
    x_t_full[D + 1] = 1.0
    x_t_full = x_t_full.astype(bf16)
    # W1hat rows: 0:64 = W1[1:], 64 = W1[0] (t row), 65 = b1 (ones row)
    w1h = np.concatenate([W1[1:], W1[0:1], b1[None, :]], axis=0).astype(bf16)
    # G for the divergence bilinear form, scaled 2^8 into fp8e4m3's
    # normal range (entries ~2e-3 would otherwise be denormal)
    gmat = W2 * (W1[1:].T @ W3.T)
    # device weight bundle [128, 4*512 + 4*576]: W2 tiles, then [G | W3]
    w24_np = W2.reshape(KT, 128, H).transpose(1, 0, 2)
    gw3_np = np.concatenate(
        [
            gmat.reshape(KT, 128, H).transpose(1, 0, 2),
            W3.reshape(KT, 128, D).transpose(1, 0, 2),
        ],
        axis=2,
    )
    wgw = np.concatenate(
        [w24_np.reshape(128, KT * H), gw3_np.reshape(128, KT * (H + D))], axis=1
    ).astype(bf16)
    b3a = np.zeros((D + 1, 1), np.float32)
    b3a[0:D, 0] = b3

    base = {
        "wgw_d": np.ascontiguousarray(wgw),
        "b2v": b2,
        "b3a": b3a,
    }
    return [
        dict(
            base,
            xw_d=np.ascontiguousarray(
                np.concatenate(
                    [w1h, x_t_full[:, i * BC : (i + 1) * BC]], axis=1
                )
            ),
        )
        for i in range(N_CORES)
    ]


def kernel(t, x, W1, b1, W2, b2, W3, b3):
    from concourse.bass_utils import run_bass_kernel_spmd

    nc = _get_nc()
    in_maps = _make_in_maps(t, x, W1, b1, W2, b2, W3, b3)
    res = run_bass_kernel_spmd(nc, in_maps, core_ids=list(range(N_CORES)))
    _CACHE["last_result"] = res
    return np.ascontiguousarray(
        np.concatenate(
            [res.results[i]["out_t"].T for i in range(N_CORES)], axis=0
        )
    )


# revision 59
# speedup vs baseline: 1.0104x; 1.0104x over previous
"""CNF vector field + exact divergence kernel for Trainium2 (8 NeuronCores).

Math (per sample x of dim D=64, t scalar, 3-layer MLP 65->512->512->64):
    h1 = tanh(W1^T [t;x] + b1)
    h2 = tanh(W2^T h1 + b2)
    dx = W3^T h2 + b3
    div = trace(d dx / d x) collapses to the bilinear form
        div = (1-h1^2)^T G (1-h2^2)   with  G = W2 * (W1[1:].T @ W3.T)

Sharding: pure data parallel over batch; 8192/8 = 1024 samples per core;
weights replicated.  Everything on device is feature-major ([feature
partitions, batch free]): the host pre-transposes x (and appends t/ones
rows), precomputes G, and re-transposes the feature-major output, so the
device program is matmul/activation/elementwise only — no PE transposes,
no identity, no PSUM round-trips beyond the matmul outputs themselves.

Per-core device work (CH=batch chunk):
    L1  : h1 = tanh(W1hat^T xhat)            (b1/t folded into xhat rows)
    L2  : h2 = tanh(W2^T h1 + b2)
    c   : c  = G^T s1m,  s1m = (h1-1)(h1+1) = -(1-h1^2)
    e   : e  = (s2q-1) * c = (1-h2^2) * (G^T (1-h1^2)),  s2q = h2^2
    L3  : out[0:64] = W3^T h2 (+b3);  out[64] = ones^T e = div
    out : one [65, CH] DMA per chunk, host transposes back
"""

import sys

if "/opt/trn_rl_repo" not in sys.path:
    sys.path.insert(0, "/opt/trn_rl_repo")

import numpy as np

D = 64
H = 512
B = 8192
N_CORES = 8
BC = B // N_CORES          # 1024 samples per core
NCH = 4                    # batch chunks per core (pipeline granularity)
CH = BC // NCH             # 256 moving columns per matmul
KT = H // 128              # 4 k-tiles of the hidden dim
N_WARM = 18                # PE warm-up matmuls during the DMA head (~3.8us
                           # sustained at the cold clock, to flip HAM to 8/8)
# A=L1, B=L2, C=c/ebuf, L=L3, V=div+act+dma  (per chunk 0-3)
# all four L1 stages run first: 16 matmuls bridge the PE exactly until the
# w24 DMA-complete sem (~14.7us), so HAM never re-throttles mid-kernel
SCHEDULE = [
    "A0", "A1", "A2", "A3", "B0", "B1", "L0", "C0", "B2", "L1",
    "C1", "V0", "B3", "L2", "C2", "V1", "C3", "L3", "V2", "V3",
]

_CACHE = {}


def _patch_tile_drain():
    """walrus in this toolchain accepts only one sync wait per CTRL
    instruction; split the TileContext tail-drain waits across nops."""
    import concourse.mybir as mybir
    from concourse.tile import TileContext
    from concourse.vector_clock import ScopedClock

    if getattr(TileContext, "_drain_patched", False):
        return

    def _drain_and_barrier(self, tick_clock, wait_clock):
        nc = self.nc
        probe = nc.sync.nop(nofuse=True, hint="drain_wait_probe")
        wait_clock.add_sem_waits(
            probe.ins, ScopedClock({None: tick_clock.global_clock})
        )
        waits = list(probe.ins.sync_info.on_wait) if probe.ins.sync_info else []
        if len(waits) > 1:
            probe.ins.sync_info.on_wait.clear()
            probe.ins.sync_info.on_wait.append(waits[0])
            for w in waits[1:]:
                nop_inst = nc.sync.nop(nofuse=True, hint="drain_wait_split")
                if nop_inst.ins.sync_info is None:
                    nop_inst.ins.sync_info = mybir.SyncInfo(on_wait=[], on_update=[])
                nop_inst.ins.sync_info.on_wait.append(w)
        nc.sync.drain()  # SP already observed every sem above
        popped = nc._tile_sem_poison_stack.pop()
        assert popped is self._sem_poison
        # sem clears skipped: NRT reloads sem state per execution; verified
        # by repeated-call correctness checks in test.py

    TileContext._orig_drain_and_barrier = TileContext._drain_and_barrier
    TileContext._drain_and_barrier = _drain_and_barrier
    TileContext._drain_patched = True


# this walrus build has small per-instruction sync-wait budgets; split any
# excess waits onto same-engine nops placed just before the instruction
# (waiting earlier on the same engine stream is always safe).
_WAIT_LIMITS = {"DMACOPY": 1, "NOOP": 1, "DRAIN": 1, "TRIGGEREDCOPY": 1}
_DEFAULT_WAIT_LIMIT = 1


def _split_excess_waits(nc):
    import concourse.mybir as mybir

    ctr = 0
    for fn in nc.m.functions:
        for blk in fn.blocks:
            lst = blk.instructions
            out = []
            changed = False
            for inst in lst:
                si = inst.sync_info
                waits = list(si.on_wait) if si else []
                opname = type(inst).__name__.replace("Inst", "").upper()
                limit = _WAIT_LIMITS.get(opname, _DEFAULT_WAIT_LIMIT)
                if len(waits) > limit:
                    keep = waits[-limit:]
                    excess = waits[:-limit]
                    si.on_wait.clear()
                    for w in keep:
                        si.on_wait.append(w)
                    for w in excess:
                        nop = mybir.InstNoOp(name=f"WSPLIT-{ctr}", ins=[], outs=[])
                        ctr += 1
                        nop.engine = inst.engine
                        nop.sync_info = mybir.SyncInfo(on_wait=[w], on_update=[])
                        out.append(nop)
                    changed = True
                out.append(inst)
            if changed:
                lst[:] = out


def _build(for_sim=False):
    import concourse.bass as bass
    import concourse.mybir as mybir
    from concourse.tile import TileContext

    _patch_tile_drain()

    f32 = mybir.dt.float32
    bf16 = mybir.dt.bfloat16
    f8 = mybir.dt.float8e4
    AF = mybir.ActivationFunctionType
    OP = mybir.AluOpType

    nc = bass.Bass(trn_type="TRN2")
    nc._bass_sim_build = for_sim

    # host-prepped inputs (see kernel()); all matmul operands in bf16
    # (fp32 PSUM accumulation keeps the error ~0.4%, well under the 2e-2
    # budget), fp32 only for biases and the final output:
    #   xw_d  : [66, 512+BC]  cols 0:512 = W1hat (rows: W1[1:], W1[0], b1),
    #                         cols 512:  = x^T with t/ones rows appended
    #   wgw_d : [128, 4*512 + 4*576]  = W2 tiles, then [G | W3] tiles,
    #                         each [p, t, m] flattened (p=128, t=4)
    #   b3a   : [65, 1]       rows 0:64 = b3, row 64 = 0
    xw_d = nc.dram_tensor("xw_d", [D + 2, H + BC], bf16, kind="ExternalInput")
    wgw_d = nc.dram_tensor(
        "wgw_d", [128, KT * H + KT * (H + D)], bf16, kind="ExternalInput"
    )
    b2v = nc.dram_tensor("b2v", [H], f32, kind="ExternalInput")
    b3a = nc.dram_tensor("b3a", [D + 1, 1], f32, kind="ExternalInput")
    out_t = nc.dram_tensor("out_t", [D + 1, BC], f32, kind="ExternalOutput")

    with TileContext(nc) as tc:
        with (
            tc.tile_pool(name="weights", bufs=1) as wpool,
            tc.tile_pool(name="acts", bufs=1) as apool,
            tc.tile_pool(name="psmm", bufs=6, space="PSUM") as psmm,
            tc.tile_pool(name="pso", bufs=2, space="PSUM") as pso,
        ):
            ones_col = wpool.tile([128, 1], bf16)
            nc.vector.memset(ones_col, 1.0)
            warm_mv = wpool.tile([128, 256], bf16)
            nc.vector.memset(warm_mv, 0.0)

            # ---------------- input / weight DMAs --------------------------
            # strict need-order on one queue so each transfer gets the full
            # parallel DMA-engine bandwidth: xw (L1) -> w24 (L2) -> g4/w34 (c).
            # A concurrent w24 on the other queue delays xw past the warm-up
            # dummies and the resulting PE gap re-throttles HAM.
            xw = apool.tile([D + 2, H + BC], bf16)
            nc.sync.dma_start(out=xw, in_=xw_d[:])
            w1hat = xw[:, 0:H]
            xhat = xw[:, H : H + BC]
            wgw = wpool.tile([128, KT, 2 * H + D], bf16)
            nc.sync.dma_start(
                out=wgw[:, :, 0:H],
                in_=wgw_d[:, 0 : KT * H].rearrange("p (t m) -> p t m", t=KT),
            )
            nc.sync.dma_start(
                out=wgw[:, :, H:],
                in_=wgw_d[:, KT * H :].rearrange("p (t m) -> p t m", t=KT),
            )
            w24 = wgw[:, :, 0:H]
            g4 = wgw[:, :, H : 2 * H]
            w34 = wgw[:, :, 2 * H : 2 * H + D]
            b2t = wpool.tile([128, KT], f32)
            nc.gpsimd.dma_start(out=b2t, in_=b2v[:].rearrange("(t p) -> p t", p=128))
            b3t = wpool.tile([D + 1, 1], f32)
            nc.gpsimd.dma_start(out=b3t, in_=b3a[:])

            # ---------------- PE warm-up during the DMA head ---------------
            # HAM un-throttles the PE clock only after ~3.4us of sustained
            # activity; burn the DMA wait on dummy matmuls so the real ones
            # start at full rate.
            pwarm = psmm.tile([1, 256], f32, tag="mmtile", name="pwarm")
            for _ in range(N_WARM):
                nc.tensor.matmul(pwarm, ones_col, warm_mv, start=True, stop=True)

            # per-chunk activation tiles (bf16 matmul operands)
            h1 = [apool.tile([128, KT, CH], bf16, tag=f"h1_{n}", name=f"h1_{n}") for n in range(NCH)]
            u1 = [apool.tile([128, KT, CH], bf16, tag=f"u1_{n}", name=f"u1_{n}") for n in range(NCH)]
            s1m = [apool.tile([128, KT, CH], bf16, tag=f"s1m{n}", name=f"s1m{n}") for n in range(NCH)]
            h2 = [apool.tile([128, KT, CH], bf16, tag=f"h2_{n}", name=f"h2_{n}") for n in range(NCH)]
            s2q = [apool.tile([128, KT, CH], bf16, tag=f"s2q{n}", name=f"s2q{n}") for n in range(NCH)]
            ebuf = [apool.tile([128, KT, CH], bf16, tag=f"eb{n}", name=f"eb{n}") for n in range(NCH)]

            # ---- stage emitters (per chunk) -------------------------------
            def stage_l1(n):
                # h1 = tanh(W1hat^T xhat); u1 = h1+1; s1m = (h1-1)(h1+1)
                xh_n = xhat[:, n * CH : (n + 1) * CH]
                for i in range(KT):
                    pz = psmm.tile([128, CH], f32, tag="mmtile")
                    nc.tensor.matmul(
                        pz,
                        w1hat[:, i * 128 : (i + 1) * 128],
                        xh_n,
                        start=True,
                        stop=True,
                    )
                    nc.scalar.activation(h1[n][:, i, :], pz, AF.Tanh)
                    nc.vector.tensor_scalar_add(u1[n][:, i, :], h1[n][:, i, :], 1.0)
                    nc.vector.scalar_tensor_tensor(
                        out=s1m[n][:, i, :],
                        in0=h1[n][:, i, :],
                        scalar=1.0,
                        in1=u1[n][:, i, :],
                        op0=OP.subtract,
                        op1=OP.mult,
                    )

            def stage_l2(n, k_outer=False):
                # h2 = tanh(W2^T h1 + b2); s2q = h2^2
                # k_outer: first 4 matmuls need only h1[k=0] — pipelines with
                # the serialized tanh chain during the L1(0) fill phase.
                pzs = [
                    psmm.tile([128, CH], f32, tag="mmtile", name=f"pz{n}_{i}")
                    for i in range(KT)
                ]
                loop = (
                    [(i, k) for k in range(KT) for i in range(KT)]
                    if k_outer
                    else [(i, k) for i in range(KT) for k in range(KT)]
                )
                for i, k in loop:
                    nc.tensor.matmul(
                        pzs[i],
                        w24[:, k, i * 128 : (i + 1) * 128],
                        h1[n][:, k, :],
                        start=(k == 0),
                        stop=(k == KT - 1),
                    )
                    if k == KT - 1:
                        nc.scalar.activation(
                            h2[n][:, i, :], pzs[i], AF.Tanh, bias=b2t[:, i : i + 1]
                        )
                        nc.gpsimd.tensor_mul(
                            s2q[n][:, i, :], h2[n][:, i, :], h2[n][:, i, :]
                        )

            def stage_c(n):
                # c = G^T s1m; e = (s2q - 1) * c
                for i in range(KT):
                    pc = psmm.tile([128, CH], f32, tag="mmtile")
                    for k in range(KT):
                        nc.tensor.matmul(
                            pc,
                            g4[:, k, i * 128 : (i + 1) * 128],
                            s1m[n][:, k, :],
                            start=(k == 0),
                            stop=(k == KT - 1),
                        )
                    # gpsimd cannot read PSUM; ebuf stays on DVE
                    nc.vector.scalar_tensor_tensor(
                        out=ebuf[n][:, i, :],
                        in0=s2q[n][:, i, :],
                        scalar=1.0,
                        in1=pc,
                        op0=OP.subtract,
                        op1=OP.mult,
                    )

            # po tiles live from stage_l3 (L3 matmuls, needs only h2) until
            # stage_div (div matmuls + act + DMA, needs ebuf)
            po_t = [None] * NCH
            ott = apool.tile([D + 1, BC], f32, name="ott")

            def stage_l3(n):
                # out[0:64] = W3^T h2 (b3 added by the Identity act later)
                po = pso.tile([D + 1, CH], f32, tag="po", name=f"po{n}")
                po_t[n] = po
                for k in range(KT):
                    nc.tensor.matmul(
                        po[0:D, :],
                        w34[:, k, :],
                        h2[n][:, k, :],
                        start=(k == 0),
                        stop=(k == KT - 1),
                    )

            def stage_div(n):
                # out[64] = ones^T e, then one act + one DMA out
                po = po_t[n]
                for k in range(KT):
                    nc.tensor.matmul(
                        po[D : D + 1, :],
                        ones_col,
                        ebuf[n][:, k, :],
                        start=(k == 0),
                        stop=(k == KT - 1),
                    )
                nc.scalar.activation(ott[:, n * CH : (n + 1) * CH], po, AF.Identity, bias=b3t)
                if n % 2 == 1:
                    # one DMA per chunk pair, alternating HWDGE queues so the
                    # last two dispatches overlap instead of serializing
                    dq = nc.scalar if n == 1 else nc.sync
                    lo, hi = (n - 1) * CH, (n + 1) * CH
                    dq.dma_start(out=out_t[:, lo:hi], in_=ott[:, lo:hi])

            # ---- software-pipeline schedule over chunks -------------------
            # Explicit order; each engine's stream interleaves across chunks
            # so no serialized elementwise stream gates a later consumer.
            # A=L1, B=L2, C=c/ebuf, L=L3, V=div+act+dma.
            emit = {
                "A": stage_l1,
                "B": stage_l2,
                "C": stage_c,
                "L": stage_l3,
                "V": stage_div,
            }
            for step in SCHEDULE:
                if step[0] == "W":
                    # filler dummies: keep PE busy/warm while a DMA lands
                    for _ in range(int(step[1:])):
                        nc.tensor.matmul(
                            pwarm, ones_col, warm_mv, start=True, stop=True
                        )
                elif step == "B0":
                    stage_l2(0, k_outer=True)
                else:
                    emit[step[0]](int(step[1]))

    if not for_sim:
        _split_excess_waits(nc)
    return nc


def _get_nc():
    if "nc" not in _CACHE:
        _CACHE["nc"] = _build()
    return _CACHE["nc"]


def _make_in_maps(t, x, W1, b1, W2, b2, W3, b3):
    t = np.asarray(t, np.float32)
    x = np.asarray(x, np.float32)
    W1 = np.asarray(W1, np.float32)
    b1 = np.asarray(b1, np.float32)
    W2 = np.asarray(W2, np.float32)
    b2 = np.asarray(b2, np.float32)
    W3 = np.asarray(W3, np.float32)
    b3 = np.asarray(b3, np.float32)

    import ml_dtypes

    bf16 = ml_dtypes.bfloat16
    fp8 = ml_dtypes.float8_e4m3

    # feature-major input with t/ones rows appended: [66, B]
    x_t_full = np.empty((D + 2, B), np.float32)
    x_t_full[0:D] = x[:, 0:D].T
    x_t_full[D] = t[0]
    x_t_full[D + 1] = 1.0
    x_t_full = x_t_full.astype(bf16)
    # W1hat rows: 0:64 = W1[1:], 64 = W1[0] (t row), 65 = b1 (ones row)
    w1h = np.concatenate([W1[1:], W1[0:1], b1[None, :]], axis=0).astype(bf16)
    # G for the divergence bilinear form, scaled 2^8 into fp8e4m3's
    # normal range (entries ~2e-3 would otherwise be denormal)
    gmat = W2 * (W1[1:].T @ W3.T)
    # device weight bundle [128, 4*512 + 4*576]: W2 tiles, then [G | W3]
    w24_np = W2.reshape(KT, 128, H).transpose(1, 0, 2)
    gw3_np = np.concatenate(
        [
            gmat.reshape(KT, 128, H).transpose(1, 0, 2),
            W3.reshape(KT, 128, D).transpose(1, 0, 2),
        ],
        axis=2,
    )
    wgw = np.concatenate(
        [w24_np.reshape(128, KT * H), gw3_np.reshape(128, KT * (H + D))], axis=1
    ).astype(bf16)
    b3a = np.zeros((D + 1, 1), np.float32)
    b3a[0:D, 0] = b3

    base = {
        "wgw_d": np.ascontiguousarray(wgw),
        "b2v": b2,
        "b3a": b3a,
    }
    return [
        dict(
            base,
            xw_d=np.ascontiguousarray(
                np.concatenate(
                    [w1h, x_t_full[:, i * BC : (i + 1) * BC]], axis=1
                )
            ),
        )
        for i in range(N_CORES)
    ]


def kernel(t, x, W1, b1, W2, b2, W3, b3):
    from concourse.bass_utils import run_bass_kernel_spmd

    nc = _get_nc()
    in_maps = _make_in_maps(t, x, W1, b1, W2, b2, W3, b3)
    res = run_bass_kernel_spmd(nc, in_maps, core_ids=list(range(N_CORES)))
    _CACHE["last_result"] = res
    return np.ascontiguousarray(
        np.concatenate(
            [res.results[i]["out_t"].T for i in range(N_CORES)], axis=0
        )
    )


# revision 60
# speedup vs baseline: 1.0428x; 1.0321x over previous
"""CNF vector field + exact divergence kernel for Trainium2 (8 NeuronCores).

Math (per sample x of dim D=64, t scalar, 3-layer MLP 65->512->512->64):
    h1 = tanh(W1^T [t;x] + b1)
    h2 = tanh(W2^T h1 + b2)
    dx = W3^T h2 + b3
    div = trace(d dx / d x) collapses to the bilinear form
        div = (1-h1^2)^T G (1-h2^2)   with  G = W2 * (W1[1:].T @ W3.T)

Sharding: pure data parallel over batch; 8192/8 = 1024 samples per core;
weights replicated.  Everything on device is feature-major ([feature
partitions, batch free]): the host pre-transposes x (and appends t/ones
rows), precomputes G, and re-transposes the feature-major output, so the
device program is matmul/activation/elementwise only — no PE transposes,
no identity, no PSUM round-trips beyond the matmul outputs themselves.

Per-core device work (CH=batch chunk):
    L1  : h1 = tanh(W1hat^T xhat)            (b1/t folded into xhat rows)
    L2  : h2 = tanh(W2^T h1 + b2)
    c   : c  = G^T s1m,  s1m = (h1-1)(h1+1) = -(1-h1^2)
    e   : e  = (s2q-1) * c = (1-h2^2) * (G^T (1-h1^2)),  s2q = h2^2
    L3  : out[0:64] = W3^T h2 (+b3);  out[64] = ones^T e = div
    out : one [65, CH] DMA per chunk, host transposes back
"""

import sys

if "/opt/trn_rl_repo" not in sys.path:
    sys.path.insert(0, "/opt/trn_rl_repo")

import numpy as np

D = 64
H = 512
B = 8192
N_CORES = 8
BC = B // N_CORES          # 1024 samples per core
NCH = 4                    # batch chunks per core (pipeline granularity)
CH = BC // NCH             # 256 moving columns per matmul
KT = H // 128              # 4 k-tiles of the hidden dim
N_WARM = 13                # PE warm-up matmuls during the DMA head: bridge
                           # until the xw DMA sem; L1 then continues the busy
                           # streak so HAM still flips to 8/8 at ~3.4us
# A=L1, B=L2, C=c/ebuf, L=L3, V=div+act+dma  (per chunk 0-3)
# all four L1 stages run first: 16 matmuls bridge the PE exactly until the
# w24 DMA-complete sem (~14.7us), so HAM never re-throttles mid-kernel
SCHEDULE = [
    "A0", "A1", "A2", "A3", "B0", "B1", "L0", "C0", "B2", "L1",
    "C1", "V0", "B3", "L2", "C2", "V1", "C3", "L3", "V2", "V3",
]

_CACHE = {}


def _patch_tile_drain():
    """walrus in this toolchain accepts only one sync wait per CTRL
    instruction; split the TileContext tail-drain waits across nops."""
    import concourse.mybir as mybir
    from concourse.tile import TileContext
    from concourse.vector_clock import ScopedClock

    if getattr(TileContext, "_drain_patched", False):
        return

    def _drain_and_barrier(self, tick_clock, wait_clock):
        nc = self.nc
        probe = nc.sync.nop(nofuse=True, hint="drain_wait_probe")
        wait_clock.add_sem_waits(
            probe.ins, ScopedClock({None: tick_clock.global_clock})
        )
        waits = list(probe.ins.sync_info.on_wait) if probe.ins.sync_info else []
        if len(waits) > 1:
            probe.ins.sync_info.on_wait.clear()
            probe.ins.sync_info.on_wait.append(waits[0])
            for w in waits[1:]:
                nop_inst = nc.sync.nop(nofuse=True, hint="drain_wait_split")
                if nop_inst.ins.sync_info is None:
                    nop_inst.ins.sync_info = mybir.SyncInfo(on_wait=[], on_update=[])
                nop_inst.ins.sync_info.on_wait.append(w)
        nc.sync.drain()  # SP already observed every sem above
        popped = nc._tile_sem_poison_stack.pop()
        assert popped is self._sem_poison
        # sem clears skipped: NRT reloads sem state per execution; verified
        # by repeated-call correctness checks in test.py

    TileContext._orig_drain_and_barrier = TileContext._drain_and_barrier
    TileContext._drain_and_barrier = _drain_and_barrier
    TileContext._drain_patched = True


# this walrus build has small per-instruction sync-wait budgets; split any
# excess waits onto same-engine nops placed just before the instruction
# (waiting earlier on the same engine stream is always safe).
_WAIT_LIMITS = {"DMACOPY": 1, "NOOP": 1, "DRAIN": 1, "TRIGGEREDCOPY": 1}
_DEFAULT_WAIT_LIMIT = 1


def _split_excess_waits(nc):
    import concourse.mybir as mybir

    ctr = 0
    for fn in nc.m.functions:
        for blk in fn.blocks:
            lst = blk.instructions
            out = []
            changed = False
            for inst in lst:
                si = inst.sync_info
                waits = list(si.on_wait) if si else []
                opname = type(inst).__name__.replace("Inst", "").upper()
                limit = _WAIT_LIMITS.get(opname, _DEFAULT_WAIT_LIMIT)
                if len(waits) > limit:
                    keep = waits[-limit:]
                    excess = waits[:-limit]
                    si.on_wait.clear()
                    for w in keep:
                        si.on_wait.append(w)
                    for w in excess:
                        nop = mybir.InstNoOp(name=f"WSPLIT-{ctr}", ins=[], outs=[])
                        ctr += 1
                        nop.engine = inst.engine
                        nop.sync_info = mybir.SyncInfo(on_wait=[w], on_update=[])
                        out.append(nop)
                    changed = True
                out.append(inst)
            if changed:
                lst[:] = out


def _build(for_sim=False):
    import concourse.bass as bass
    import concourse.mybir as mybir
    from concourse.tile import TileContext

    _patch_tile_drain()

    f32 = mybir.dt.float32
    bf16 = mybir.dt.bfloat16
    f8 = mybir.dt.float8e4
    AF = mybir.ActivationFunctionType
    OP = mybir.AluOpType

    nc = bass.Bass(trn_type="TRN2")
    nc._bass_sim_build = for_sim

    # host-prepped inputs (see kernel()); all matmul operands in bf16
    # (fp32 PSUM accumulation keeps the error ~0.4%, well under the 2e-2
    # budget), fp32 only for biases and the final output:
    #   xw_d  : [66, 512+BC]  cols 0:512 = W1hat (rows: W1[1:], W1[0], b1),
    #                         cols 512:  = x^T with t/ones rows appended
    #   wgw_d : [128, 4*512 + 4*576]  = W2 tiles, then [G | W3] tiles,
    #                         each [p, t, m] flattened (p=128, t=4)
    #   b3a   : [65, 1]       rows 0:64 = b3, row 64 = 0
    xw_d = nc.dram_tensor("xw_d", [D + 2, H + BC], bf16, kind="ExternalInput")
    wgw_d = nc.dram_tensor(
        "wgw_d", [128, KT * H + KT * (H + D)], bf16, kind="ExternalInput"
    )
    b2v = nc.dram_tensor("b2v", [H], f32, kind="ExternalInput")
    b3a = nc.dram_tensor("b3a", [D + 1, 1], f32, kind="ExternalInput")
    out_t = nc.dram_tensor("out_t", [D + 1, BC], f32, kind="ExternalOutput")

    with TileContext(nc) as tc:
        with (
            tc.tile_pool(name="weights", bufs=1) as wpool,
            tc.tile_pool(name="acts", bufs=1) as apool,
            tc.tile_pool(name="psmm", bufs=6, space="PSUM") as psmm,
            tc.tile_pool(name="pso", bufs=2, space="PSUM") as pso,
        ):
            ones_col = wpool.tile([128, 1], bf16)
            nc.vector.memset(ones_col, 1.0)
            warm_mv = wpool.tile([128, 256], bf16)
            nc.vector.memset(warm_mv, 0.0)

            # ---------------- input / weight DMAs --------------------------
            # strict need-order on one queue so each transfer gets the full
            # parallel DMA-engine bandwidth: xw (L1) -> w24 (L2) -> g4/w34 (c).
            # A concurrent w24 on the other queue delays xw past the warm-up
            # dummies and the resulting PE gap re-throttles HAM.
            xw = apool.tile([D + 2, H + BC], bf16)
            nc.sync.dma_start(out=xw, in_=xw_d[:])
            w1hat = xw[:, 0:H]
            xhat = xw[:, H : H + BC]
            wgw = wpool.tile([128, KT, 2 * H + D], bf16)
            # w24 split by k-tile pairs: region tracking lets the k-outer
            # L2(0) start on tiles 0-1 ~1us before the full matrix lands
            nc.sync.dma_start(
                out=wgw[:, 0:2, 0:H],
                in_=wgw_d[:, 0 : 2 * H].rearrange("p (t m) -> p t m", t=2),
            )
            nc.sync.dma_start(
                out=wgw[:, 2:KT, 0:H],
                in_=wgw_d[:, 2 * H : KT * H].rearrange("p (t m) -> p t m", t=2),
            )
            nc.sync.dma_start(
                out=wgw[:, :, H:],
                in_=wgw_d[:, KT * H :].rearrange("p (t m) -> p t m", t=KT),
            )
            w24 = wgw[:, :, 0:H]
            g4 = wgw[:, :, H : 2 * H]
            w34 = wgw[:, :, 2 * H : 2 * H + D]
            b2t = wpool.tile([128, KT], f32)
            nc.gpsimd.dma_start(out=b2t, in_=b2v[:].rearrange("(t p) -> p t", p=128))
            b3t = wpool.tile([D + 1, 1], f32)
            nc.gpsimd.dma_start(out=b3t, in_=b3a[:])

            # ---------------- PE warm-up during the DMA head ---------------
            # HAM un-throttles the PE clock only after ~3.4us of sustained
            # activity; burn the DMA wait on dummy matmuls so the real ones
            # start at full rate.
            pwarm = psmm.tile([1, 256], f32, tag="mmtile", name="pwarm")
            for _ in range(N_WARM):
                nc.tensor.matmul(pwarm, ones_col, warm_mv, start=True, stop=True)

            # per-chunk activation tiles (bf16 matmul operands)
            h1 = [apool.tile([128, KT, CH], bf16, tag=f"h1_{n}", name=f"h1_{n}") for n in range(NCH)]
            u1 = [apool.tile([128, KT, CH], bf16, tag=f"u1_{n}", name=f"u1_{n}") for n in range(NCH)]
            s1m = [apool.tile([128, KT, CH], bf16, tag=f"s1m{n}", name=f"s1m{n}") for n in range(NCH)]
            h2 = [apool.tile([128, KT, CH], bf16, tag=f"h2_{n}", name=f"h2_{n}") for n in range(NCH)]
            s2q = [apool.tile([128, KT, CH], bf16, tag=f"s2q{n}", name=f"s2q{n}") for n in range(NCH)]
            ebuf = [apool.tile([128, KT, CH], bf16, tag=f"eb{n}", name=f"eb{n}") for n in range(NCH)]

            # ---- stage emitters (per chunk) -------------------------------
            def stage_l1(n):
                # h1 = tanh(W1hat^T xhat); u1 = h1+1; s1m = (h1-1)(h1+1)
                xh_n = xhat[:, n * CH : (n + 1) * CH]
                for i in range(KT):
                    pz = psmm.tile([128, CH], f32, tag="mmtile")
                    nc.tensor.matmul(
                        pz,
                        w1hat[:, i * 128 : (i + 1) * 128],
                        xh_n,
                        start=True,
                        stop=True,
                    )
                    nc.scalar.activation(h1[n][:, i, :], pz, AF.Tanh)
                    nc.vector.tensor_scalar_add(u1[n][:, i, :], h1[n][:, i, :], 1.0)
                    nc.vector.scalar_tensor_tensor(
                        out=s1m[n][:, i, :],
                        in0=h1[n][:, i, :],
                        scalar=1.0,
                        in1=u1[n][:, i, :],
                        op0=OP.subtract,
                        op1=OP.mult,
                    )

            def stage_l2(n, k_outer=False):
                # h2 = tanh(W2^T h1 + b2); s2q = h2^2
                # k_outer: first 4 matmuls need only h1[k=0] — pipelines with
                # the serialized tanh chain during the L1(0) fill phase.
                pzs = [
                    psmm.tile([128, CH], f32, tag="mmtile", name=f"pz{n}_{i}")
                    for i in range(KT)
                ]
                loop = (
                    [(i, k) for k in range(KT) for i in range(KT)]
                    if k_outer
                    else [(i, k) for i in range(KT) for k in range(KT)]
                )
                for i, k in loop:
                    nc.tensor.matmul(
                        pzs[i],
                        w24[:, k, i * 128 : (i + 1) * 128],
                        h1[n][:, k, :],
                        start=(k == 0),
                        stop=(k == KT - 1),
                    )
                    if k == KT - 1:
                        nc.scalar.activation(
                            h2[n][:, i, :], pzs[i], AF.Tanh, bias=b2t[:, i : i + 1]
                        )
                        nc.gpsimd.tensor_mul(
                            s2q[n][:, i, :], h2[n][:, i, :], h2[n][:, i, :]
                        )

            def stage_c(n):
                # c = G^T s1m; e = (s2q - 1) * c
                for i in range(KT):
                    pc = psmm.tile([128, CH], f32, tag="mmtile")
                    for k in range(KT):
                        nc.tensor.matmul(
                            pc,
                            g4[:, k, i * 128 : (i + 1) * 128],
                            s1m[n][:, k, :],
                            start=(k == 0),
                            stop=(k == KT - 1),
                        )
                    # gpsimd cannot read PSUM; ebuf stays on DVE
                    nc.vector.scalar_tensor_tensor(
                        out=ebuf[n][:, i, :],
                        in0=s2q[n][:, i, :],
                        scalar=1.0,
                        in1=pc,
                        op0=OP.subtract,
                        op1=OP.mult,
                    )

            # po tiles live from stage_l3 (L3 matmuls, needs only h2) until
            # stage_div (div matmuls + act + DMA, needs ebuf)
            po_t = [None] * NCH
            ott = apool.tile([D + 1, BC], f32, name="ott")

            def stage_l3(n):
                # out[0:64] = W3^T h2 (b3 added by the Identity act later)
                po = pso.tile([D + 1, CH], f32, tag="po", name=f"po{n}")
                po_t[n] = po
                for k in range(KT):
                    nc.tensor.matmul(
                        po[0:D, :],
                        w34[:, k, :],
                        h2[n][:, k, :],
                        start=(k == 0),
                        stop=(k == KT - 1),
                    )

            def stage_div(n):
                # out[64] = ones^T e, then one act + one DMA out
                po = po_t[n]
                for k in range(KT):
                    nc.tensor.matmul(
                        po[D : D + 1, :],
                        ones_col,
                        ebuf[n][:, k, :],
                        start=(k == 0),
                        stop=(k == KT - 1),
                    )
                nc.scalar.activation(ott[:, n * CH : (n + 1) * CH], po, AF.Identity, bias=b3t)
                if n % 2 == 1:
                    # one DMA per chunk pair, alternating HWDGE queues so the
                    # last two dispatches overlap instead of serializing
                    dq = nc.scalar if n == 1 else nc.sync
                    lo, hi = (n - 1) * CH, (n + 1) * CH
                    dq.dma_start(out=out_t[:, lo:hi], in_=ott[:, lo:hi])

            # ---- software-pipeline schedule over chunks -------------------
            # Explicit order; each engine's stream interleaves across chunks
            # so no serialized elementwise stream gates a later consumer.
            # A=L1, B=L2, C=c/ebuf, L=L3, V=div+act+dma.
            emit = {
                "A": stage_l1,
                "B": stage_l2,
                "C": stage_c,
                "L": stage_l3,
                "V": stage_div,
            }
            for step in SCHEDULE:
                if step[0] == "W":
                    # filler dummies: keep PE busy/warm while a DMA lands
                    for _ in range(int(step[1:])):
                        nc.tensor.matmul(
                            pwarm, ones_col, warm_mv, start=True, stop=True
                        )
                elif step == "B0":
                    stage_l2(0, k_outer=True)
                else:
                    emit[step[0]](int(step[1]))

    if not for_sim:
        _split_excess_waits(nc)
    return nc


def _get_nc():
    if "nc" not in _CACHE:
        _CACHE["nc"] = _build()
    return _CACHE["nc"]


def _make_in_maps(t, x, W1, b1, W2, b2, W3, b3):
    t = np.asarray(t, np.float32)
    x = np.asarray(x, np.float32)
    W1 = np.asarray(W1, np.float32)
    b1 = np.asarray(b1, np.float32)
    W2 = np.asarray(W2, np.float32)
    b2 = np.asarray(b2, np.float32)
    W3 = np.asarray(W3, np.float32)
    b3 = np.asarray(b3, np.float32)

    import ml_dtypes

    bf16 = ml_dtypes.bfloat16
    fp8 = ml_dtypes.float8_e4m3

    # feature-major input with t/ones rows appended: [66, B]
    x_t_full = np.empty((D + 2, B), np.float32)
    x_t_full[0:D] = x[:, 0:D].T
    x_t_full[D] = t[0]
    x_t_full[D + 1] = 1.0
    x_t_full = x_t_full.astype(bf16)
    # W1hat rows: 0:64 = W1[1:], 64 = W1[0] (t row), 65 = b1 (ones row)
    w1h = np.concatenate([W1[1:], W1[0:1], b1[None, :]], axis=0).astype(bf16)
    # G for the divergence bilinear form, scaled 2^8 into fp8e4m3's
    # normal range (entries ~2e-3 would otherwise be denormal)
    gmat = W2 * (W1[1:].T @ W3.T)
    # device weight bundle [128, 4*512 + 4*576]: W2 tiles, then [G | W3]
    w24_np = W2.reshape(KT, 128, H).transpose(1, 0, 2)
    gw3_np = np.concatenate(
        [
            gmat.reshape(KT, 128, H).transpose(1, 0, 2),
            W3.reshape(KT, 128, D).transpose(1, 0, 2),
        ],
        axis=2,
    )
    wgw = np.concatenate(
        [w24_np.reshape(128, KT * H), gw3_np.reshape(128, KT * (H + D))], axis=1
    ).astype(bf16)
    b3a = np.zeros((D + 1, 1), np.float32)
    b3a[0:D, 0] = b3

    base = {
        "wgw_d": np.ascontiguousarray(wgw),
        "b2v": b2,
        "b3a": b3a,
    }
    return [
        dict(
            base,
            xw_d=np.ascontiguousarray(
                np.concatenate(
                    [w1h, x_t_full[:, i * BC : (i + 1) * BC]], axis=1
                )
            ),
        )
        for i in range(N_CORES)
    ]


def kernel(t, x, W1, b1, W2, b2, W3, b3):
    from concourse.bass_utils import run_bass_kernel_spmd

    nc = _get_nc()
    in_maps = _make_in_maps(t, x, W1, b1, W2, b2, W3, b3)
    res = run_bass_kernel_spmd(nc, in_maps, core_ids=list(range(N_CORES)))
    _CACHE["last_result"] = res
    return np.ascontiguousarray(
        np.concatenate(
            [res.results[i]["out_t"].T for i in range(N_CORES)], axis=0
        )
    )


# revision 61
# speedup vs baseline: 1.0455x; 1.0025x over previous
"""CNF vector field + exact divergence kernel for Trainium2 (8 NeuronCores).

Math (per sample x of dim D=64, t scalar, 3-layer MLP 65->512->512->64):
    h1 = tanh(W1^T [t;x] + b1)
    h2 = tanh(W2^T h1 + b2)
    dx = W3^T h2 + b3
    div = trace(d dx / d x) collapses to the bilinear form
        div = (1-h1^2)^T G (1-h2^2)   with  G = W2 * (W1[1:].T @ W3.T)

Sharding: pure data parallel over batch; 8192/8 = 1024 samples per core;
weights replicated.  Everything on device is feature-major ([feature
partitions, batch free]): the host pre-transposes x (and appends t/ones
rows), precomputes G, and re-transposes the feature-major output, so the
device program is matmul/activation/elementwise only — no PE transposes,
no identity, no PSUM round-trips beyond the matmul outputs themselves.

Per-core device work (CH=batch chunk):
    L1  : h1 = tanh(W1hat^T xhat)            (b1/t folded into xhat rows)
    L2  : h2 = tanh(W2^T h1 + b2)
    c   : c  = G^T s1m,  s1m = (h1-1)(h1+1) = -(1-h1^2)
    e   : e  = (s2q-1) * c = (1-h2^2) * (G^T (1-h1^2)),  s2q = h2^2
    L3  : out[0:64] = W3^T h2 (+b3);  out[64] = ones^T e = div
    out : one [65, CH] DMA per chunk, host transposes back
"""

import sys

if "/opt/trn_rl_repo" not in sys.path:
    sys.path.insert(0, "/opt/trn_rl_repo")

import numpy as np

D = 64
H = 512
B = 8192
N_CORES = 8
BC = B // N_CORES          # 1024 samples per core
NCH = 4                    # batch chunks per core (pipeline granularity)
CH = BC // NCH             # 256 moving columns per matmul
KT = H // 128              # 4 k-tiles of the hidden dim
N_WARM = 15                # PE warm-up matmuls during the DMA head: bridge
                           # until the xw DMA sem; L1 then continues the busy
                           # streak so HAM still flips to 8/8 at ~3.4us
# A=L1, B=L2, C=c/ebuf, L=L3, V=div+act+dma  (per chunk 0-3)
# all four L1 stages run first: 16 matmuls bridge the PE exactly until the
# w24 DMA-complete sem (~14.7us), so HAM never re-throttles mid-kernel
SCHEDULE = [
    "A0", "A1", "A2", "A3", "B0", "B1", "L0", "C0", "B2", "L1",
    "C1", "V0", "B3", "L2", "C2", "V1", "C3", "L3", "V2", "V3",
]

_CACHE = {}


def _patch_tile_drain():
    """walrus in this toolchain accepts only one sync wait per CTRL
    instruction; split the TileContext tail-drain waits across nops."""
    import concourse.mybir as mybir
    from concourse.tile import TileContext
    from concourse.vector_clock import ScopedClock

    if getattr(TileContext, "_drain_patched", False):
        return

    def _drain_and_barrier(self, tick_clock, wait_clock):
        nc = self.nc
        probe = nc.sync.nop(nofuse=True, hint="drain_wait_probe")
        wait_clock.add_sem_waits(
            probe.ins, ScopedClock({None: tick_clock.global_clock})
        )
        waits = list(probe.ins.sync_info.on_wait) if probe.ins.sync_info else []
        if len(waits) > 1:
            probe.ins.sync_info.on_wait.clear()
            probe.ins.sync_info.on_wait.append(waits[0])
            for w in waits[1:]:
                nop_inst = nc.sync.nop(nofuse=True, hint="drain_wait_split")
                if nop_inst.ins.sync_info is None:
                    nop_inst.ins.sync_info = mybir.SyncInfo(on_wait=[], on_update=[])
                nop_inst.ins.sync_info.on_wait.append(w)
        nc.sync.drain()  # SP already observed every sem above
        popped = nc._tile_sem_poison_stack.pop()
        assert popped is self._sem_poison
        # sem clears skipped: NRT reloads sem state per execution; verified
        # by repeated-call correctness checks in test.py

    TileContext._orig_drain_and_barrier = TileContext._drain_and_barrier
    TileContext._drain_and_barrier = _drain_and_barrier
    TileContext._drain_patched = True


# this walrus build has small per-instruction sync-wait budgets; split any
# excess waits onto same-engine nops placed just before the instruction
# (waiting earlier on the same engine stream is always safe).
_WAIT_LIMITS = {"DMACOPY": 1, "NOOP": 1, "DRAIN": 1, "TRIGGEREDCOPY": 1}
_DEFAULT_WAIT_LIMIT = 1


def _split_excess_waits(nc):
    import concourse.mybir as mybir

    ctr = 0
    for fn in nc.m.functions:
        for blk in fn.blocks:
            lst = blk.instructions
            out = []
            changed = False
            for inst in lst:
                si = inst.sync_info
                waits = list(si.on_wait) if si else []
                opname = type(inst).__name__.replace("Inst", "").upper()
                limit = _WAIT_LIMITS.get(opname, _DEFAULT_WAIT_LIMIT)
                if len(waits) > limit:
                    keep = waits[-limit:]
                    excess = waits[:-limit]
                    si.on_wait.clear()
                    for w in keep:
                        si.on_wait.append(w)
                    for w in excess:
                        nop = mybir.InstNoOp(name=f"WSPLIT-{ctr}", ins=[], outs=[])
                        ctr += 1
                        nop.engine = inst.engine
                        nop.sync_info = mybir.SyncInfo(on_wait=[w], on_update=[])
                        out.append(nop)
                    changed = True
                out.append(inst)
            if changed:
                lst[:] = out


def _build(for_sim=False):
    import concourse.bass as bass
    import concourse.mybir as mybir
    from concourse.tile import TileContext

    _patch_tile_drain()

    f32 = mybir.dt.float32
    bf16 = mybir.dt.bfloat16
    f8 = mybir.dt.float8e4
    AF = mybir.ActivationFunctionType
    OP = mybir.AluOpType

    nc = bass.Bass(trn_type="TRN2")
    nc._bass_sim_build = for_sim

    # host-prepped inputs (see kernel()); all matmul operands in bf16
    # (fp32 PSUM accumulation keeps the error ~0.4%, well under the 2e-2
    # budget), fp32 only for biases and the final output:
    #   xw_d  : [66, 512+BC]  cols 0:512 = W1hat (rows: W1[1:], W1[0], b1),
    #                         cols 512:  = x^T with t/ones rows appended
    #   wgw_d : [128, 4*512 + 4*576]  = W2 tiles, then [G | W3] tiles,
    #                         each [p, t, m] flattened (p=128, t=4)
    #   b3a   : [65, 1]       rows 0:64 = b3, row 64 = 0
    xw_d = nc.dram_tensor("xw_d", [D + 2, H + BC], bf16, kind="ExternalInput")
    wgw_d = nc.dram_tensor(
        "wgw_d", [128, KT * H + KT * (H + D)], bf16, kind="ExternalInput"
    )
    b2v = nc.dram_tensor("b2v", [H], f32, kind="ExternalInput")
    b3a = nc.dram_tensor("b3a", [D + 1, 1], f32, kind="ExternalInput")
    out_t = nc.dram_tensor("out_t", [D + 1, BC], f32, kind="ExternalOutput")

    with TileContext(nc) as tc:
        with (
            tc.tile_pool(name="weights", bufs=1) as wpool,
            tc.tile_pool(name="acts", bufs=1) as apool,
            tc.tile_pool(name="psmm", bufs=6, space="PSUM") as psmm,
            tc.tile_pool(name="pso", bufs=2, space="PSUM") as pso,
        ):
            ones_col = wpool.tile([128, 1], bf16)
            nc.vector.memset(ones_col, 1.0)
            warm_mv = wpool.tile([128, 256], bf16)
            nc.vector.memset(warm_mv, 0.0)

            # ---------------- input / weight DMAs --------------------------
            # strict need-order on one queue so each transfer gets the full
            # parallel DMA-engine bandwidth: xw (L1) -> w24 (L2) -> g4/w34 (c).
            # A concurrent w24 on the other queue delays xw past the warm-up
            # dummies and the resulting PE gap re-throttles HAM.
            xw = apool.tile([D + 2, H + BC], bf16)
            nc.sync.dma_start(out=xw, in_=xw_d[:])
            w1hat = xw[:, 0:H]
            xhat = xw[:, H : H + BC]
            wgw = wpool.tile([128, KT, 2 * H + D], bf16)
            # w24 split by k-tile pairs: region tracking lets the k-outer
            # L2(0) start on tiles 0-1 ~1us before the full matrix lands
            nc.sync.dma_start(
                out=wgw[:, 0:2, 0:H],
                in_=wgw_d[:, 0 : 2 * H].rearrange("p (t m) -> p t m", t=2),
            )
            nc.sync.dma_start(
                out=wgw[:, 2:KT, 0:H],
                in_=wgw_d[:, 2 * H : KT * H].rearrange("p (t m) -> p t m", t=2),
            )
            nc.sync.dma_start(
                out=wgw[:, :, H:],
                in_=wgw_d[:, KT * H :].rearrange("p (t m) -> p t m", t=KT),
            )
            w24 = wgw[:, :, 0:H]
            g4 = wgw[:, :, H : 2 * H]
            w34 = wgw[:, :, 2 * H : 2 * H + D]
            b2t = wpool.tile([128, KT], f32)
            nc.gpsimd.dma_start(out=b2t, in_=b2v[:].rearrange("(t p) -> p t", p=128))
            b3t = wpool.tile([D + 1, 1], f32)
            nc.gpsimd.dma_start(out=b3t, in_=b3a[:])

            # ---------------- PE warm-up during the DMA head ---------------
            # HAM un-throttles the PE clock only after ~3.4us of sustained
            # activity; burn the DMA wait on dummy matmuls so the real ones
            # start at full rate.
            pwarm = psmm.tile([1, 256], f32, tag="mmtile", name="pwarm")
            for _ in range(N_WARM):
                nc.tensor.matmul(pwarm, ones_col, warm_mv, start=True, stop=True)

            # per-chunk activation tiles (bf16 matmul operands)
            h1 = [apool.tile([128, KT, CH], bf16, tag=f"h1_{n}", name=f"h1_{n}") for n in range(NCH)]
            u1 = [apool.tile([128, KT, CH], bf16, tag=f"u1_{n}", name=f"u1_{n}") for n in range(NCH)]
            s1m = [apool.tile([128, KT, CH], bf16, tag=f"s1m{n}", name=f"s1m{n}") for n in range(NCH)]
            h2 = [apool.tile([128, KT, CH], bf16, tag=f"h2_{n}", name=f"h2_{n}") for n in range(NCH)]
            s2q = [apool.tile([128, KT, CH], bf16, tag=f"s2q{n}", name=f"s2q{n}") for n in range(NCH)]
            ebuf = [apool.tile([128, KT, CH], bf16, tag=f"eb{n}", name=f"eb{n}") for n in range(NCH)]

            # ---- stage emitters (per chunk) -------------------------------
            def stage_l1(n):
                # h1 = tanh(W1hat^T xhat); u1 = h1+1; s1m = (h1-1)(h1+1)
                xh_n = xhat[:, n * CH : (n + 1) * CH]
                for i in range(KT):
                    pz = psmm.tile([128, CH], f32, tag="mmtile")
                    nc.tensor.matmul(
                        pz,
                        w1hat[:, i * 128 : (i + 1) * 128],
                        xh_n,
                        start=True,
                        stop=True,
                    )
                    nc.scalar.activation(h1[n][:, i, :], pz, AF.Tanh)
                    nc.vector.tensor_scalar_add(u1[n][:, i, :], h1[n][:, i, :], 1.0)
                    nc.vector.scalar_tensor_tensor(
                        out=s1m[n][:, i, :],
                        in0=h1[n][:, i, :],
                        scalar=1.0,
                        in1=u1[n][:, i, :],
                        op0=OP.subtract,
                        op1=OP.mult,
                    )

            def stage_l2(n, k_outer=False):
                # h2 = tanh(W2^T h1 + b2); s2q = h2^2
                # k_outer: first 4 matmuls need only h1[k=0] — pipelines with
                # the serialized tanh chain during the L1(0) fill phase.
                pzs = [
                    psmm.tile([128, CH], f32, tag="mmtile", name=f"pz{n}_{i}")
                    for i in range(KT)
                ]
                loop = (
                    [(i, k) for k in range(KT) for i in range(KT)]
                    if k_outer
                    else [(i, k) for i in range(KT) for k in range(KT)]
                )
                for i, k in loop:
                    nc.tensor.matmul(
                        pzs[i],
                        w24[:, k, i * 128 : (i + 1) * 128],
                        h1[n][:, k, :],
                        start=(k == 0),
                        stop=(k == KT - 1),
                    )
                    if k == KT - 1:
                        nc.scalar.activation(
                            h2[n][:, i, :], pzs[i], AF.Tanh, bias=b2t[:, i : i + 1]
                        )
                        nc.gpsimd.tensor_mul(
                            s2q[n][:, i, :], h2[n][:, i, :], h2[n][:, i, :]
                        )

            def stage_c(n):
                # c = G^T s1m; e = (s2q - 1) * c
                for i in range(KT):
                    pc = psmm.tile([128, CH], f32, tag="mmtile")
                    for k in range(KT):
                        nc.tensor.matmul(
                            pc,
                            g4[:, k, i * 128 : (i + 1) * 128],
                            s1m[n][:, k, :],
                            start=(k == 0),
                            stop=(k == KT - 1),
                        )
                    # gpsimd cannot read PSUM; ebuf stays on DVE
                    nc.vector.scalar_tensor_tensor(
                        out=ebuf[n][:, i, :],
                        in0=s2q[n][:, i, :],
                        scalar=1.0,
                        in1=pc,
                        op0=OP.subtract,
                        op1=OP.mult,
                    )

            # po tiles live from stage_l3 (L3 matmuls, needs only h2) until
            # stage_div (div matmuls + act + DMA, needs ebuf)
            po_t = [None] * NCH
            ott = apool.tile([D + 1, BC], f32, name="ott")

            def stage_l3(n):
                # out[0:64] = W3^T h2 (b3 added by the Identity act later)
                po = pso.tile([D + 1, CH], f32, tag="po", name=f"po{n}")
                po_t[n] = po
                for k in range(KT):
                    nc.tensor.matmul(
                        po[0:D, :],
                        w34[:, k, :],
                        h2[n][:, k, :],
                        start=(k == 0),
                        stop=(k == KT - 1),
                    )

            def stage_div(n):
                # out[64] = ones^T e, then one act + one DMA out
                po = po_t[n]
                for k in range(KT):
                    nc.tensor.matmul(
                        po[D : D + 1, :],
                        ones_col,
                        ebuf[n][:, k, :],
                        start=(k == 0),
                        stop=(k == KT - 1),
                    )
                nc.scalar.activation(ott[:, n * CH : (n + 1) * CH], po, AF.Identity, bias=b3t)
                if n % 2 == 1:
                    # one DMA per chunk pair, alternating HWDGE queues so the
                    # last two dispatches overlap instead of serializing
                    dq = nc.scalar if n == 1 else nc.sync
                    lo, hi = (n - 1) * CH, (n + 1) * CH
                    dq.dma_start(out=out_t[:, lo:hi], in_=ott[:, lo:hi])

            # ---- software-pipeline schedule over chunks -------------------
            # Explicit order; each engine's stream interleaves across chunks
            # so no serialized elementwise stream gates a later consumer.
            # A=L1, B=L2, C=c/ebuf, L=L3, V=div+act+dma.
            emit = {
                "A": stage_l1,
                "B": stage_l2,
                "C": stage_c,
                "L": stage_l3,
                "V": stage_div,
            }
            for step in SCHEDULE:
                if step[0] == "W":
                    # filler dummies: keep PE busy/warm while a DMA lands
                    for _ in range(int(step[1:])):
                        nc.tensor.matmul(
                            pwarm, ones_col, warm_mv, start=True, stop=True
                        )
                elif step == "B0":
                    stage_l2(0, k_outer=True)
                else:
                    emit[step[0]](int(step[1]))

    if not for_sim:
        _split_excess_waits(nc)
    return nc


def _get_nc():
    if "nc" not in _CACHE:
        _CACHE["nc"] = _build()
    return _CACHE["nc"]


def _make_in_maps(t, x, W1, b1, W2, b2, W3, b3):
    t = np.asarray(t, np.float32)
    x = np.asarray(x, np.float32)
    W1 = np.asarray(W1, np.float32)
    b1 = np.asarray(b1, np.float32)
    W2 = np.asarray(W2, np.float32)
    b2 = np.asarray(b2, np.float32)
    W3 = np.asarray(W3, np.float32)
    b3 = np.asarray(b3, np.float32)

    import ml_dtypes

    bf16 = ml_dtypes.bfloat16
    fp8 = ml_dtypes.float8_e4m3

    # feature-major input with t/ones rows appended: [66, B]
    x_t_full = np.empty((D + 2, B), np.float32)
    x_t_full[0:D] = x[:, 0:D].T
    x_t_full[D] = t[0]
    x_t_full[D + 1] = 1.0
    x_t_full = x_t_full.astype(bf16)
    # W1hat rows: 0:64 = W1[1:], 64 = W1[0] (t row), 65 = b1 (ones row)
    w1h = np.concatenate([W1[1:], W1[0:1], b1[None, :]], axis=0).astype(bf16)
    # G for the divergence bilinear form, scaled 2^8 into fp8e4m3's
    # normal range (entries ~2e-3 would otherwise be denormal)
    gmat = W2 * (W1[1:].T @ W3.T)
    # device weight bundle [128, 4*512 + 4*576]: W2 tiles, then [G | W3]
    w24_np = W2.reshape(KT, 128, H).transpose(1, 0, 2)
    gw3_np = np.concatenate(
        [
            gmat.reshape(KT, 128, H).transpose(1, 0, 2),
            W3.reshape(KT, 128, D).transpose(1, 0, 2),
        ],
        axis=2,
    )
    wgw = np.concatenate(
        [w24_np.reshape(128, KT * H), gw3_np.reshape(128, KT * (H + D))], axis=1
    ).astype(bf16)
    b3a = np.zeros((D + 1, 1), np.float32)
    b3a[0:D, 0] = b3

    base = {
        "wgw_d": np.ascontiguousarray(wgw),
        "b2v": b2,
        "b3a": b3a,
    }
    return [
        dict(
            base,
            xw_d=np.ascontiguousarray(
                np.concatenate(
                    [w1h, x_t_full[:, i * BC : (i + 1) * BC]], axis=1
                )
            ),
        )
        for i in range(N_CORES)
    ]


def kernel(t, x, W1, b1, W2, b2, W3, b3):
    from concourse.bass_utils import run_bass_kernel_spmd

    nc = _get_nc()
    in_maps = _make_in_maps(t, x, W1, b1, W2, b2, W3, b3)
    res = run_bass_kernel_spmd(nc, in_maps, core_ids=list(range(N_CORES)))
    _CACHE["last_result"] = res
    return np.ascontiguousarray(
        np.concatenate(
            [res.results[i]["out_t"].T for i in range(N_CORES)], axis=0
        )
    )


# revision 62
# speedup vs baseline: 1.0583x; 1.0123x over previous
"""CNF vector field + exact divergence kernel for Trainium2 (8 NeuronCores).

Math (per sample x of dim D=64, t scalar, 3-layer MLP 65->512->512->64):
    h1 = tanh(W1^T [t;x] + b1)
    h2 = tanh(W2^T h1 + b2)
    dx = W3^T h2 + b3
    div = trace(d dx / d x) collapses to the bilinear form
        div = (1-h1^2)^T G (1-h2^2)   with  G = W2 * (W1[1:].T @ W3.T)

Sharding: pure data parallel over batch; 8192/8 = 1024 samples per core;
weights replicated.  Everything on device is feature-major ([feature
partitions, batch free]): the host pre-transposes x (and appends t/ones
rows), precomputes G, and re-transposes the feature-major output, so the
device program is matmul/activation/elementwise only — no PE transposes,
no identity, no PSUM round-trips beyond the matmul outputs themselves.

Per-core device work (CH=batch chunk):
    L1  : h1 = tanh(W1hat^T xhat)            (b1/t folded into xhat rows)
    L2  : h2 = tanh(W2^T h1 + b2)
    c   : c  = G^T s1m,  s1m = (h1-1)(h1+1) = -(1-h1^2)
    e   : e  = (s2q-1) * c = (1-h2^2) * (G^T (1-h1^2)),  s2q = h2^2
    L3  : out[0:64] = W3^T h2 (+b3);  out[64] = ones^T e = div
    out : one [65, CH] DMA per chunk, host transposes back
"""

import sys

if "/opt/trn_rl_repo" not in sys.path:
    sys.path.insert(0, "/opt/trn_rl_repo")

import numpy as np

D = 64
H = 512
B = 8192
N_CORES = 8
BC = B // N_CORES          # 1024 samples per core
NCH = 4                    # batch chunks per core (pipeline granularity)
CH = BC // NCH             # 256 moving columns per matmul
KT = H // 128              # 4 k-tiles of the hidden dim
N_WARM = 15                # PE warm-up matmuls during the DMA head: bridge
                           # until the xw DMA sem; L1 then continues the busy
                           # streak so HAM still flips to 8/8 at ~3.4us
# A=L1, B=L2, C=c/ebuf, L=L3, V=div+act+dma  (per chunk 0-3)
# all four L1 stages run first: 16 matmuls bridge the PE exactly until the
# w24 DMA-complete sem (~14.7us), so HAM never re-throttles mid-kernel
# W-fillers hold ~100% PE duty through the fill phase so HAM un-throttles
# before the dense L2/c block instead of ~7us into real work
SCHEDULE = [
    "A0", "W2", "A1", "W2", "A2", "W2", "A3", "W4", "B0", "B1",
    "L0", "C0", "B2", "L1", "C1", "V0", "B3", "L2", "C2", "V1",
    "C3", "L3", "V2", "V3",
]

_CACHE = {}


def _patch_tile_drain():
    """walrus in this toolchain accepts only one sync wait per CTRL
    instruction; split the TileContext tail-drain waits across nops."""
    import concourse.mybir as mybir
    from concourse.tile import TileContext
    from concourse.vector_clock import ScopedClock

    if getattr(TileContext, "_drain_patched", False):
        return

    def _drain_and_barrier(self, tick_clock, wait_clock):
        nc = self.nc
        probe = nc.sync.nop(nofuse=True, hint="drain_wait_probe")
        wait_clock.add_sem_waits(
            probe.ins, ScopedClock({None: tick_clock.global_clock})
        )
        waits = list(probe.ins.sync_info.on_wait) if probe.ins.sync_info else []
        if len(waits) > 1:
            probe.ins.sync_info.on_wait.clear()
            probe.ins.sync_info.on_wait.append(waits[0])
            for w in waits[1:]:
                nop_inst = nc.sync.nop(nofuse=True, hint="drain_wait_split")
                if nop_inst.ins.sync_info is None:
                    nop_inst.ins.sync_info = mybir.SyncInfo(on_wait=[], on_update=[])
                nop_inst.ins.sync_info.on_wait.append(w)
        nc.sync.drain()  # SP already observed every sem above
        popped = nc._tile_sem_poison_stack.pop()
        assert popped is self._sem_poison
        # sem clears skipped: NRT reloads sem state per execution; verified
        # by repeated-call correctness checks in test.py

    TileContext._orig_drain_and_barrier = TileContext._drain_and_barrier
    TileContext._drain_and_barrier = _drain_and_barrier
    TileContext._drain_patched = True


# this walrus build has small per-instruction sync-wait budgets; split any
# excess waits onto same-engine nops placed just before the instruction
# (waiting earlier on the same engine stream is always safe).
_WAIT_LIMITS = {"DMACOPY": 1, "NOOP": 1, "DRAIN": 1, "TRIGGEREDCOPY": 1}
_DEFAULT_WAIT_LIMIT = 1


def _split_excess_waits(nc):
    import concourse.mybir as mybir

    ctr = 0
    for fn in nc.m.functions:
        for blk in fn.blocks:
            lst = blk.instructions
            out = []
            changed = False
            for inst in lst:
                si = inst.sync_info
                waits = list(si.on_wait) if si else []
                opname = type(inst).__name__.replace("Inst", "").upper()
                limit = _WAIT_LIMITS.get(opname, _DEFAULT_WAIT_LIMIT)
                if len(waits) > limit:
                    keep = waits[-limit:]
                    excess = waits[:-limit]
                    si.on_wait.clear()
                    for w in keep:
                        si.on_wait.append(w)
                    for w in excess:
                        nop = mybir.InstNoOp(name=f"WSPLIT-{ctr}", ins=[], outs=[])
                        ctr += 1
                        nop.engine = inst.engine
                        nop.sync_info = mybir.SyncInfo(on_wait=[w], on_update=[])
                        out.append(nop)
                    changed = True
                out.append(inst)
            if changed:
                lst[:] = out


def _build(for_sim=False):
    import concourse.bass as bass
    import concourse.mybir as mybir
    from concourse.tile import TileContext

    _patch_tile_drain()

    f32 = mybir.dt.float32
    bf16 = mybir.dt.bfloat16
    f8 = mybir.dt.float8e4
    AF = mybir.ActivationFunctionType
    OP = mybir.AluOpType

    nc = bass.Bass(trn_type="TRN2")
    nc._bass_sim_build = for_sim

    # host-prepped inputs (see kernel()); all matmul operands in bf16
    # (fp32 PSUM accumulation keeps the error ~0.4%, well under the 2e-2
    # budget), fp32 only for biases and the final output:
    #   xw_d  : [66, 512+BC]  cols 0:512 = W1hat (rows: W1[1:], W1[0], b1),
    #                         cols 512:  = x^T with t/ones rows appended
    #   wgw_d : [128, 4*512 + 4*576]  = W2 tiles, then [G | W3] tiles,
    #                         each [p, t, m] flattened (p=128, t=4)
    #   b3a   : [65, 1]       rows 0:64 = b3, row 64 = 0
    xw_d = nc.dram_tensor("xw_d", [D + 2, H + BC], bf16, kind="ExternalInput")
    wgw_d = nc.dram_tensor(
        "wgw_d", [128, KT * H + KT * (H + D)], bf16, kind="ExternalInput"
    )
    b2v = nc.dram_tensor("b2v", [H], f32, kind="ExternalInput")
    b3a = nc.dram_tensor("b3a", [D + 1, 1], f32, kind="ExternalInput")
    out_t = nc.dram_tensor("out_t", [D + 1, BC], f32, kind="ExternalOutput")

    with TileContext(nc) as tc:
        with (
            tc.tile_pool(name="weights", bufs=1) as wpool,
            tc.tile_pool(name="acts", bufs=1) as apool,
            tc.tile_pool(name="psmm", bufs=6, space="PSUM") as psmm,
            tc.tile_pool(name="pso", bufs=2, space="PSUM") as pso,
        ):
            ones_col = wpool.tile([128, 1], bf16)
            nc.vector.memset(ones_col, 1.0)
            warm_mv = wpool.tile([128, 256], bf16)
            nc.vector.memset(warm_mv, 0.0)

            # ---------------- input / weight DMAs --------------------------
            # strict need-order on one queue so each transfer gets the full
            # parallel DMA-engine bandwidth: xw (L1) -> w24 (L2) -> g4/w34 (c).
            # A concurrent w24 on the other queue delays xw past the warm-up
            # dummies and the resulting PE gap re-throttles HAM.
            xw = apool.tile([D + 2, H + BC], bf16)
            nc.sync.dma_start(out=xw, in_=xw_d[:])
            w1hat = xw[:, 0:H]
            xhat = xw[:, H : H + BC]
            wgw = wpool.tile([128, KT, 2 * H + D], bf16)
            # w24 split by k-tile pairs: region tracking lets the k-outer
            # L2(0) start on tiles 0-1 ~1us before the full matrix lands
            nc.sync.dma_start(
                out=wgw[:, 0:2, 0:H],
                in_=wgw_d[:, 0 : 2 * H].rearrange("p (t m) -> p t m", t=2),
            )
            nc.sync.dma_start(
                out=wgw[:, 2:KT, 0:H],
                in_=wgw_d[:, 2 * H : KT * H].rearrange("p (t m) -> p t m", t=2),
            )
            nc.sync.dma_start(
                out=wgw[:, :, H:],
                in_=wgw_d[:, KT * H :].rearrange("p (t m) -> p t m", t=KT),
            )
            w24 = wgw[:, :, 0:H]
            g4 = wgw[:, :, H : 2 * H]
            w34 = wgw[:, :, 2 * H : 2 * H + D]
            b2t = wpool.tile([128, KT], f32)
            nc.gpsimd.dma_start(out=b2t, in_=b2v[:].rearrange("(t p) -> p t", p=128))
            b3t = wpool.tile([D + 1, 1], f32)
            nc.gpsimd.dma_start(out=b3t, in_=b3a[:])

            # ---------------- PE warm-up during the DMA head ---------------
            # HAM un-throttles the PE clock only after ~3.4us of sustained
            # activity; burn the DMA wait on dummy matmuls so the real ones
            # start at full rate.
            pwarm = psmm.tile([1, 256], f32, tag="mmtile", name="pwarm")
            for _ in range(N_WARM):
                nc.tensor.matmul(pwarm, ones_col, warm_mv, start=True, stop=True)

            # per-chunk activation tiles (bf16 matmul operands)
            h1 = [apool.tile([128, KT, CH], bf16, tag=f"h1_{n}", name=f"h1_{n}") for n in range(NCH)]
            u1 = [apool.tile([128, KT, CH], bf16, tag=f"u1_{n}", name=f"u1_{n}") for n in range(NCH)]
            s1m = [apool.tile([128, KT, CH], bf16, tag=f"s1m{n}", name=f"s1m{n}") for n in range(NCH)]
            h2 = [apool.tile([128, KT, CH], bf16, tag=f"h2_{n}", name=f"h2_{n}") for n in range(NCH)]
            s2q = [apool.tile([128, KT, CH], bf16, tag=f"s2q{n}", name=f"s2q{n}") for n in range(NCH)]
            ebuf = [apool.tile([128, KT, CH], bf16, tag=f"eb{n}", name=f"eb{n}") for n in range(NCH)]

            # ---- stage emitters (per chunk) -------------------------------
            def stage_l1(n):
                # h1 = tanh(W1hat^T xhat); u1 = h1+1; s1m = (h1-1)(h1+1)
                xh_n = xhat[:, n * CH : (n + 1) * CH]
                for i in range(KT):
                    pz = psmm.tile([128, CH], f32, tag="mmtile")
                    nc.tensor.matmul(
                        pz,
                        w1hat[:, i * 128 : (i + 1) * 128],
                        xh_n,
                        start=True,
                        stop=True,
                    )
                    nc.scalar.activation(h1[n][:, i, :], pz, AF.Tanh)
                    nc.vector.tensor_scalar_add(u1[n][:, i, :], h1[n][:, i, :], 1.0)
                    nc.vector.scalar_tensor_tensor(
                        out=s1m[n][:, i, :],
                        in0=h1[n][:, i, :],
                        scalar=1.0,
                        in1=u1[n][:, i, :],
                        op0=OP.subtract,
                        op1=OP.mult,
                    )

            def stage_l2(n, k_outer=False):
                # h2 = tanh(W2^T h1 + b2); s2q = h2^2
                # k_outer: first 4 matmuls need only h1[k=0] — pipelines with
                # the serialized tanh chain during the L1(0) fill phase.
                pzs = [
                    psmm.tile([128, CH], f32, tag="mmtile", name=f"pz{n}_{i}")
                    for i in range(KT)
                ]
                loop = (
                    [(i, k) for k in range(KT) for i in range(KT)]
                    if k_outer
                    else [(i, k) for i in range(KT) for k in range(KT)]
                )
                for i, k in loop:
                    nc.tensor.matmul(
                        pzs[i],
                        w24[:, k, i * 128 : (i + 1) * 128],
                        h1[n][:, k, :],
                        start=(k == 0),
                        stop=(k == KT - 1),
                    )
                    if k == KT - 1:
                        nc.scalar.activation(
                            h2[n][:, i, :], pzs[i], AF.Tanh, bias=b2t[:, i : i + 1]
                        )
                        nc.gpsimd.tensor_mul(
                            s2q[n][:, i, :], h2[n][:, i, :], h2[n][:, i, :]
                        )

            def stage_c(n):
                # c = G^T s1m; e = (s2q - 1) * c
                for i in range(KT):
                    pc = psmm.tile([128, CH], f32, tag="mmtile")
                    for k in range(KT):
                        nc.tensor.matmul(
                            pc,
                            g4[:, k, i * 128 : (i + 1) * 128],
                            s1m[n][:, k, :],
                            start=(k == 0),
                            stop=(k == KT - 1),
                        )
                    # gpsimd cannot read PSUM; ebuf stays on DVE
                    nc.vector.scalar_tensor_tensor(
                        out=ebuf[n][:, i, :],
                        in0=s2q[n][:, i, :],
                        scalar=1.0,
                        in1=pc,
                        op0=OP.subtract,
                        op1=OP.mult,
                    )

            # po tiles live from stage_l3 (L3 matmuls, needs only h2) until
            # stage_div (div matmuls + act + DMA, needs ebuf)
            po_t = [None] * NCH
            ott = apool.tile([D + 1, BC], f32, name="ott")

            def stage_l3(n):
                # out[0:64] = W3^T h2 (b3 added by the Identity act later)
                po = pso.tile([D + 1, CH], f32, tag="po", name=f"po{n}")
                po_t[n] = po
                for k in range(KT):
                    nc.tensor.matmul(
                        po[0:D, :],
                        w34[:, k, :],
                        h2[n][:, k, :],
                        start=(k == 0),
                        stop=(k == KT - 1),
                    )

            def stage_div(n):
                # out[64] = ones^T e, then one act + one DMA out
                po = po_t[n]
                for k in range(KT):
                    nc.tensor.matmul(
                        po[D : D + 1, :],
                        ones_col,
                        ebuf[n][:, k, :],
                        start=(k == 0),
                        stop=(k == KT - 1),
                    )
                nc.scalar.activation(ott[:, n * CH : (n + 1) * CH], po, AF.Identity, bias=b3t)
                if n % 2 == 1:
                    # one DMA per chunk pair, alternating HWDGE queues so the
                    # last two dispatches overlap instead of serializing
                    dq = nc.scalar if n == 1 else nc.sync
                    lo, hi = (n - 1) * CH, (n + 1) * CH
                    dq.dma_start(out=out_t[:, lo:hi], in_=ott[:, lo:hi])

            # ---- software-pipeline schedule over chunks -------------------
            # Explicit order; each engine's stream interleaves across chunks
            # so no serialized elementwise stream gates a later consumer.
            # A=L1, B=L2, C=c/ebuf, L=L3, V=div+act+dma.
            emit = {
                "A": stage_l1,
                "B": stage_l2,
                "C": stage_c,
                "L": stage_l3,
                "V": stage_div,
            }
            for step in SCHEDULE:
                if step[0] == "W":
                    # filler dummies: keep PE busy/warm while a DMA lands
                    for _ in range(int(step[1:])):
                        nc.tensor.matmul(
                            pwarm, ones_col, warm_mv, start=True, stop=True
                        )
                elif step == "B0":
                    stage_l2(0, k_outer=True)
                else:
                    emit[step[0]](int(step[1]))

    if not for_sim:
        _split_excess_waits(nc)
    return nc


def _get_nc():
    if "nc" not in _CACHE:
        _CACHE["nc"] = _build()
    return _CACHE["nc"]


def _make_in_maps(t, x, W1, b1, W2, b2, W3, b3):
    t = np.asarray(t, np.float32)
    x = np.asarray(x, np.float32)
    W1 = np.asarray(W1, np.float32)
    b1 = np.asarray(b1, np.float32)
    W2 = np.asarray(W2, np.float32)
    b2 = np.asarray(b2, np.float32)
    W3 = np.asarray(W3, np.float32)
    b3 = np.asarray(b3, np.float32)

    import ml_dtypes

    bf16 = ml_dtypes.bfloat16
    fp8 = ml_dtypes.float8_e4m3

    # feature-major input with t/ones rows appended: [66, B]
    x_t_full = np.empty((D + 2, B), np.float32)
    x_t_full[0:D] = x[:, 0:D].T
    x_t_full[D] = t[0]
    x_t_full[D + 1] = 1.0
    x_t_full = x_t_full.astype(bf16)
    # W1hat rows: 0:64 = W1[1:], 64 = W1[0] (t row), 65 = b1 (ones row)
    w1h = np.concatenate([W1[1:], W1[0:1], b1[None, :]], axis=0).astype(bf16)
    # G for the divergence bilinear form, scaled 2^8 into fp8e4m3's
    # normal range (entries ~2e-3 would otherwise be denormal)
    gmat = W2 * (W1[1:].T @ W3.T)
    # device weight bundle [128, 4*512 + 4*576]: W2 tiles, then [G | W3]
    w24_np = W2.reshape(KT, 128, H).transpose(1, 0, 2)
    gw3_np = np.concatenate(
        [
            gmat.reshape(KT, 128, H).transpose(1, 0, 2),
            W3.reshape(KT, 128, D).transpose(1, 0, 2),
        ],
        axis=2,
    )
    wgw = np.concatenate(
        [w24_np.reshape(128, KT * H), gw3_np.reshape(128, KT * (H + D))], axis=1
    ).astype(bf16)
    b3a = np.zeros((D + 1, 1), np.float32)
    b3a[0:D, 0] = b3

    base = {
        "wgw_d": np.ascontiguousarray(wgw),
        "b2v": b2,
        "b3a": b3a,
    }
    return [
        dict(
            base,
            xw_d=np.ascontiguousarray(
                np.concatenate(
                    [w1h, x_t_full[:, i * BC : (i + 1) * BC]], axis=1
                )
            ),
        )
        for i in range(N_CORES)
    ]


def kernel(t, x, W1, b1, W2, b2, W3, b3):
    from concourse.bass_utils import run_bass_kernel_spmd

    nc = _get_nc()
    in_maps = _make_in_maps(t, x, W1, b1, W2, b2, W3, b3)
    res = run_bass_kernel_spmd(nc, in_maps, core_ids=list(range(N_CORES)))
    _CACHE["last_result"] = res
    return np.ascontiguousarray(
        np.concatenate(
            [res.results[i]["out_t"].T for i in range(N_CORES)], axis=0
        )
    )
